# revision 1
# baseline (speedup 1.0000x reference)
"""CapsNet forward kernel for 8 Trainium2 NeuronCores (data-parallel over batch).

Key insight: the dynamic-routing loop of this CapsNet degenerates. The routing
logits `b` start at zero, so softmax over the 10 classes is uniform; `s` is then
identical for every class, `_squash` keeps it class-independent, and the update
`b += (u @ v)^T` adds the same value to every class row. Softmax of a
class-constant tensor stays uniform, so after all 3 iterations
    v[b,f,:] = squash((1/10) * sum_n u[b,n,:])  (same for every class)
and sum_n u folds into one dense [9216 x 16] contraction. The network reduces to
    conv1(9x9) + relu -> conv3d(9x9, stride 2) + relu -> X @ Wp -> tiny tail.

Device mapping (per core, 128 samples):
  stage A: conv1 as im2col matmul, K=81 patch rows, M=128 output channels per
           half, with output channels permuted so PSUM partition m = 32*d + i
           holds channel 8i+d (the layout conv3d wants: depth-major groups).
  stage B: conv3d = 8 independent 32ch->32ch 2D convs (one per depth d), done as
           81 PSUM-accumulated shifted matmuls on a 4x4 grid of 32x32 PE tiles:
           row tile t = depth slice, col tile j = 3x3 pixel quadrant of the 6x6
           output. PSUM partitions become (quadrant, out-channel), which we
           declare to be the capsule-contraction partition layout; dig_W rows
           are permuted on the host to match, so the relu evacuation writes the
           stage-C operand directly (no data rearrangement anywhere).
  stage C: s^T[16,128] = sum over 72 accumulated [128x16]^T @ [128x128] matmuls.
  tail:    squash + 16->10 matmul + softmax, via PE transpose tricks.
All matmuls bf16 (fp32 PSUM accumulation); fp32 matmuls cost 4 cycles/row on
TRN2, bf16 1 cycle/row. End-to-end error vs fp32 reference ~2e-3 relative.
"""

import math
import sys
from contextlib import ExitStack

import numpy as np

sys.path.insert(0, "/opt/trn_rl_repo")

from ml_dtypes import bfloat16  # noqa: E402

import concourse.bass as bass  # noqa: E402
import concourse.mybir as mybir  # noqa: E402
from concourse import bacc  # noqa: E402
from concourse.ap import AP  # noqa: E402
from concourse.bass_utils import run_bass_kernel_spmd  # noqa: E402
from concourse.tile import TileContext  # noqa: E402

F32 = mybir.dt.float32
BF16 = mybir.dt.bfloat16
AF = mybir.ActivationFunctionType
ALU = mybir.AluOpType

N_CORES = 8
BD = 128          # samples per core
G = 32            # samples per chunk
N_CHUNKS = BD // G
SQRT10 = math.sqrt(10.0)


def _build_program(n_chunks=N_CHUNKS, dma_engine="sync", timing_loop=0):
    nc = bacc.Bacc("TRN2", target_bir_lowering=False, debug=False,
                   num_devices=N_CORES)

    x_d = nc.dram_tensor("x", [BD, 784], F32, kind="ExternalInput").ap()
    w1t_d = nc.dram_tensor("w1t", [81, 256], BF16, kind="ExternalInput").ap()
    b1_d = nc.dram_tensor("b1", [128, 2], F32, kind="ExternalInput").ap()
    w3_d = nc.dram_tensor("w3", [128, 2592], BF16, kind="ExternalInput").ap()
    b3_d = nc.dram_tensor("b3", [128, 1], F32, kind="ExternalInput").ap()
    wp_d = nc.dram_tensor("wp", [128, 1152], BF16, kind="ExternalInput").ap()
    sb_d = nc.dram_tensor("sb", [16, 1], F32, kind="ExternalInput").ap()
    w2t_d = nc.dram_tensor("w2t", [16, 10], F32, kind="ExternalInput").ap()
    ob_d = nc.dram_tensor("ob", [10, 1], F32, kind="ExternalInput").ap()
    ones_d = nc.dram_tensor("ones10", [10, 1], F32, kind="ExternalInput").ap()
    id_d = nc.dram_tensor("id10", [10, 10], F32, kind="ExternalInput").ap()
    y_d = nc.dram_tensor("y", [BD, 10], F32, kind="ExternalOutput").ap()

    # x rows padded to 36 cols (+ tail slack): lets the im2col gather run as 9
    # DMAs (one per ky) whose merged (y,col) inner dim is one contiguous run
    xb_dram = nc.dram_tensor("xbounce", [BD * 1008 + 1024], BF16)

    with TileContext(nc) as tc, ExitStack() as ctx:
        dma = getattr(nc, dma_engine).dma_start

        consts = ctx.enter_context(tc.tile_pool(name="consts", bufs=1))
        w1t = consts.tile([81, 256], BF16)
        b1 = consts.tile([128, 2], F32)
        w3 = consts.tile([128, 2592], BF16)
        wp = consts.tile([128, 1152], BF16)
        b3 = consts.tile([128, 1], F32)
        sbias = consts.tile([16, 1], F32)
        w2t = consts.tile([16, 10], F32)
        ob = consts.tile([10, 1], F32)
        ones10 = consts.tile([10, 1], F32)
        id10 = consts.tile([10, 10], F32)
        for t, d in [(w1t, w1t_d), (b1, b1_d), (w3, w3_d), (wp, wp_d),
                     (b3, b3_d), (sbias, sb_d), (w2t, w2t_d), (ob, ob_d),
                     (ones10, ones_d), (id10, id_d)]:
            dma(out=t[:], in_=d)

        big = ctx.enter_context(tc.tile_pool(name="big", bufs=1))
        XT = big.tile([128, 9216], BF16)          # stage-C rhs, built in place
        inp_lo = big.tile([128, G, 20, 20], BF16)  # conv1 out, depths 0-3
        inp_hi = big.tile([128, G, 20, 20], BF16)  # depths 4-7
        patch_pool = ctx.enter_context(tc.tile_pool(name="patches", bufs=2))

        # x is cast to bf16 and bounced through DRAM (per chunk) so the im2col
        # gather DMA can use a flat (overlapping-window) source access pattern
        xcast = ctx.enter_context(tc.tile_pool(name="xcast", bufs=2))

        pstack = ExitStack()
        pA = pstack.enter_context(tc.tile_pool(name="psumA", bufs=2, space="PSUM"))
        pB = pstack.enter_context(tc.tile_pool(name="psumB", bufs=1, space="PSUM"))

        dma_engines = [nc.sync, nc.gpsimd, nc.scalar]

        def chunk_body(c):
            xf = xcast.tile([G, 784], F32, tag="xf")
            xb = xcast.tile([G, 784], BF16, tag="xb")
            dma(out=xf[:], in_=x_d[c * G:(c + 1) * G, :])
            nc.scalar.activation(xb[:], xf[:], AF.Copy)
            xbp_dst = AP(tensor=xb_dram, offset=c * G * 1008,
                         ap=[[1008, G], [36, 28], [1, 28]])
            nc.gpsimd.dma_start(
                out=xbp_dst, in_=xb[:].rearrange("p (r q) -> p r q", r=28, q=28))

            # ---- im2col gather: 9 DMAs (one per ky) spread over HWDGE
            # queues; kx shift baked via the source offset -------------------
            # patch row order is p = ky + 9*kx (w1t rows permuted to match):
            # each per-ky DMA then writes partitions ky, ky+9, ..., ky+72,
            # spreading across ~8 SBUF DMA ports instead of 3
            patches = patch_pool.tile([81, G * 720], BF16, tag="patches")
            for ky in range(9):
                src = AP(tensor=xb_dram, offset=c * G * 1008 + ky * 36,
                         ap=[[1, 9], [1008, G], [1, 720]])
                dma_engines[ky % len(dma_engines)].dma_start(
                    out=patches[ky:ky + 73:9, :], in_=src)
            pview = patches[:].rearrange("p (b y q) -> p b y q", b=G, y=20, q=36)

            # ---- stage A: conv1, 2 halves x 32 samples; evacuate psum in
            # 2-sample pairs, alternating ACT/DVE ---------------------------
            for h in range(2):
                lhsT = w1t[:, h * 128:(h + 1) * 128]
                inp = inp_lo if h == 0 else inp_hi
                for s in range(0, G, 2):
                    ps = pA.tile([128, 1024], F32, tag="pa")
                    psv = ps[:].rearrange("p (two q) -> p two q", two=2, q=512)
                    nc.tensor.matmul(psv[:, 0, 0:400], lhsT,
                                     pview[:, s, :, 0:20],
                                     start=True, stop=True)
                    nc.tensor.matmul(psv[:, 1, 0:400], lhsT,
                                     pview[:, s + 1, :, 0:20],
                                     start=True, stop=True)
                    outv = inp[:, s:s + 2].rearrange("p b y x -> p b (y x)")
                    if (h * G + s) % 4 == 0:
                        nc.scalar.activation(outv, psv[:, :, 0:400], AF.Relu,
                                             bias=b1[:, h:h + 1])
                    else:
                        nc.vector.tensor_scalar(outv, psv[:, :, 0:400],
                                                b1[:, h:h + 1], 0.0,
                                                ALU.add, ALU.max)

            # ---- stage B: conv3d on 16 concurrent 32x32 PE tiles ----------
            for h in range(2):
                inp = inp_lo if h == 0 else inp_hi
                psb = pB.tile([128, 4, 512], F32, tag="pb")
                for p in range(81):
                    ky, kx = p // 9, p % 9
                    last = p == 80
                    for t in range(4):
                        lhsT = w3[32 * t:32 * t + 32, p * 32:(p + 1) * 32]
                        for j in range(4):
                            yH, xH = j // 2, j % 2
                            rhs = inp[32 * t:32 * t + 32, :,
                                      6 * yH + ky: 6 * yH + ky + 6: 2,
                                      6 * xH + kx: 6 * xH + kx + 6: 2]
                            nc.tensor.matmul(
                                psb[32 * j:32 * j + 32, t, 0:288], lhsT, rhs,
                                start=(p == 0), stop=last,
                                skip_group_check=True,
                                tile_position=(32 * t, 32 * j))
                # evacuate with relu+bias, casting to bf16, straight into XT.
                # XT free layout: [(yL,xL,d) chunk c72, b128]; this pass owns
                # d = 4h+t and samples c*G..c*G+31.
                xtv = XT[:].rearrange("p (yl xl d bb) -> p yl xl d bb",
                                      yl=3, xl=3, d=8, bb=128)
                for t in range(4):
                    srcv = psb[:, t, 0:288].rearrange(
                        "p (b yl xl) -> p b yl xl", b=G, yl=3, xl=3)
                    dstv = xtv[:, :, :, 4 * h + t,
                               c * G:(c + 1) * G].transpose([0, 3, 1, 2])
                    nc.scalar.activation(dstv, srcv, AF.Relu, bias=b3[:, 0:1])

        if timing_loop:
            for _ in range(timing_loop):
                chunk_body(0)
        else:
            for c in range(n_chunks):
                chunk_body(c)

        # ---- stage C: s^T = sum_c wp_c^T @ XT_c  (accumulate in one bank) --
        pstack.close()
        pC = ctx.enter_context(tc.tile_pool(name="psumC", bufs=1, space="PSUM"))
        ps_s = pC.tile([16, 128], F32)
        for cc in range(72):
            nc.tensor.matmul(ps_s[:], wp[:, cc * 16:(cc + 1) * 16],
                             XT[:, cc * 128:(cc + 1) * 128],
                             start=(cc == 0), stop=(cc == 71))

        # ---- tail: squash, output layer, softmax --------------------------
        tl = ctx.enter_context(tc.tile_pool(name="tail", bufs=1))
        sT = tl.tile([16, 128], F32)
        nc.vector.tensor_scalar(sT[:], ps_s[:], sbias[:, 0:1], None, ALU.add)
        aT = tl.tile([16, 128], F32)
        nc.scalar.activation(aT[:], sT[:], AF.Abs)
        dT = tl.tile([16, 128], F32)
        nc.scalar.activation(dT[:], aT[:], AF.Copy, bias=1.0, scale=SQRT10)
        rT = tl.tile([16, 128], F32)
        nc.vector.reciprocal(rT[:], dT[:])
        vT = tl.tile([16, 128], F32)
        nc.vector.tensor_tensor(vT[:], sT[:], rT[:], ALU.mult)

        ps_l = pC.tile([10, 128], F32)
        nc.tensor.matmul(ps_l[:], w2t[:], vT[:], start=True, stop=True)
        eT = tl.tile([10, 128], F32)
        nc.scalar.activation(eT[:], ps_l[:], AF.Exp, bias=ob[:, 0:1])

        ps_e = pC.tile([128, 10], F32)
        nc.tensor.matmul(ps_e[:], eT[:], id10[:], start=True, stop=True)
        ps_r = pC.tile([128, 1], F32)
        nc.tensor.matmul(ps_r[:], eT[:], ones10[:], start=True, stop=True)
        rs = tl.tile([128, 1], F32)
        nc.vector.reciprocal(rs[:], ps_r[:])
        probs = tl.tile([128, 10], F32)
        nc.vector.tensor_scalar(probs[:], ps_e[:], rs[:, 0:1], None, ALU.mult)
        dma(out=y_d, in_=probs[:])

    nc.compile()
    return nc


def _prep_weights(conv1_w, conv1_b, prim_w, prim_b, dig_W, dig_Wb, out_w, out_b):
    """Host-side weight layout preparation (all tiny)."""
    # patch rows are ordered p = ky + 9*kx on device
    perm = np.array([(p % 9) * 9 + p // 9 for p in range(81)])  # p -> ky*9+kx
    w1flat = conv1_w.reshape(256, 81)[:, perm]
    # PSUM partition m (within half h) holds conv1 channel 8i+d, d = 4h + m//32
    w1t = np.zeros((81, 256), np.float32)
    b1 = np.zeros((128, 2), np.float32)
    for h in range(2):
        for m in range(128):
            d, i = 4 * h + m // 32, m % 32
            ch = 8 * i + d
            w1t[:, h * 128 + m] = w1flat[ch]
            b1[m, h] = conv1_b[ch]

    # w3[32t+i, p*32+o] = prim_w[o,i,0,ky,kx], replicated across t
    w3c = prim_w[:, :, 0].reshape(32, 32, 81)        # [o, i, p]
    w3blk = np.transpose(w3c, (1, 2, 0))             # [i, p, o]
    w3 = np.tile(w3blk.reshape(32, 81 * 32), (4, 1)) # [128, 2592]
    b3 = np.tile(prim_b, 4)[:, None].astype(np.float32)  # [128,1]

    # wp[32j+o, cc*16+f] = 0.1*dig_W[n, d, f]
    # j=(yH,xH) quadrant, cc=(yL*3+xL)*8+d, n = o*36 + x*6 + y,
    # x = 3*xH+xL, y = 3*yH+yL  (capsule grid is [o, w=x, h=y])
    dw = dig_W.reshape(32, 6, 6, 8, 16)              # [o, x, y, d, f]
    wp = np.zeros((128, 1152), np.float32)
    for j in range(4):
        yH, xH = j // 2, j % 2
        for o in range(32):
            for yL in range(3):
                for xL in range(3):
                    for d in range(8):
                        cc = (yL * 3 + xL) * 8 + d
                        wp[32 * j + o, cc * 16:(cc + 1) * 16] = \
                            0.1 * dw[o, 3 * xH + xL, 3 * yH + yL, d]

    sb = (0.1 * dig_Wb.sum(0))[:, None].astype(np.float32)           # [16,1]
    w2t = ((1.0 / SQRT10) * out_w[..., 0].sum(1)).T.astype(np.float32)  # [16,10]
    obv = out_b[:, None].astype(np.float32)
    return dict(
        w1t=w1t.astype(bfloat16), b1=b1,
        w3=w3.astype(bfloat16), b3=b3,
        wp=wp.astype(bfloat16), sb=sb, w2t=w2t, ob=obv,
        ones10=np.ones((10, 1), np.float32),
        id10=np.eye(10, dtype=np.float32),
    )


_CACHED_NC = None


def kernel(x, conv1_w, conv1_b, prim_w, prim_b, dig_W, dig_Wb, out_w, out_b):
    global _CACHED_NC
    x = np.asarray(x, np.float32)
    wmap = _prep_weights(np.asarray(conv1_w, np.float32),
                         np.asarray(conv1_b, np.float32),
                         np.asarray(prim_w, np.float32),
                         np.asarray(prim_b, np.float32),
                         np.asarray(dig_W, np.float32),
                         np.asarray(dig_Wb, np.float32),
                         np.asarray(out_w, np.float32),
                         np.asarray(out_b, np.float32))
    if _CACHED_NC is None:
        _CACHED_NC = _build_program()
    nc = _CACHED_NC

    B = x.shape[0]
    assert B == N_CORES * BD
    xs = x.reshape(N_CORES, BD, 784)
    in_maps = [dict(x=np.ascontiguousarray(xs[i]), **wmap) for i in range(N_CORES)]
    res = run_bass_kernel_spmd(nc, in_maps, list(range(N_CORES)))
    out = np.concatenate([res.results[i]["y"] for i in range(N_CORES)], axis=0)
    return out.astype(np.float32)


if __name__ == "__main__":
    rng = np.random.default_rng(0)
    ins = dict(
        x=rng.standard_normal((1024, 1, 28, 28), dtype=np.float32),
        conv1_w=rng.standard_normal((256, 1, 9, 9), dtype=np.float32) * 0.05,
        conv1_b=np.zeros(256, np.float32),
        prim_w=rng.standard_normal((32, 32, 1, 9, 9), dtype=np.float32) * 0.05,
        prim_b=np.zeros(32, np.float32),
        dig_W=(rng.random((1152, 8, 16), dtype=np.float32) - 0.5),
        dig_Wb=np.zeros((1152, 16), np.float32),
        out_w=rng.standard_normal((10, 10, 16, 1), dtype=np.float32) * 0.05,
        out_b=np.zeros(10, np.float32),
    )
    y = kernel(**ins)
    print("out", y.shape, y.dtype, y[0])


def _build_timed_fn(nc):
    """Build the sharded jitted exec fn once (single bass_exec per module)."""
    import jax
    from jax.sharding import Mesh, PartitionSpec
    from jax.experimental.shard_map import shard_map
    import concourse.mybir as mb
    from concourse.bass2jax import _bass_exec_p, partition_id_tensor, \
        install_neuronx_cc_hook

    install_neuronx_cc_hook()
    partition_name = (nc.partition_id_tensor.name
                      if nc.partition_id_tensor else None)
    in_names, out_names, out_avals = [], [], []
    for alloc in nc.m.functions[0].allocations:
        if not isinstance(alloc, mb.MemoryLocationSet):
            continue
        name = alloc.memorylocations[0].name
        if alloc.kind == "ExternalInput":
            if name != partition_name:
                in_names.append(name)
        elif alloc.kind == "ExternalOutput":
            out_names.append(name)
            out_avals.append(jax.core.ShapedArray(
                tuple(alloc.tensor_shape), mb.dt.np(alloc.dtype)))
    n_params = len(in_names)
    n_outs = len(out_avals)
    all_names = in_names + out_names
    if partition_name is not None:
        all_names.append(partition_name)

    def _body(*args):
        operands = list(args)
        if partition_name is not None:
            operands.append(partition_id_tensor())
        return tuple(_bass_exec_p.bind(
            *operands, out_avals=tuple(out_avals),
            in_names=tuple(all_names), out_names=tuple(out_names),
            lowering_input_output_aliases=(),
            sim_require_finite=True, sim_require_nnan=True, nc=nc))

    devices = jax.devices()[:N_CORES]
    mesh = Mesh(np.asarray(devices), ("core",))
    donate = tuple(range(n_params, n_params + n_outs))
    fn = jax.jit(shard_map(_body, mesh=mesh,
                           in_specs=(PartitionSpec("core"),) * (n_params + n_outs),
                           out_specs=(PartitionSpec("core"),) * n_outs,
                           check_rep=False),
                 donate_argnums=donate, keep_unused=True)
    return fn, in_names, out_avals


def _time_calls(nc, in_maps, iters=20):
    """Wall times of repeated cached-jit exec calls (includes dispatch)."""
    import time
    import jax
    fn, in_names, out_avals = _build_timed_fn(nc)
    concat_in = [np.concatenate([np.asarray(m[name]) for m in in_maps], axis=0)
                 for name in in_names]
    concat_in = [jax.device_put(a) for a in concat_in]
    times = []
    for _ in range(iters + 1):
        zeros = [np.zeros((N_CORES * a.shape[0], *a.shape[1:]), a.dtype)
                 for a in out_avals]
        t0 = time.perf_counter()
        outs = fn(*concat_in, *zeros)
        jax.block_until_ready(outs)
        times.append(time.perf_counter() - t0)
    times = times[1:]  # drop warm-up/compile call
    times.sort()
    return times[len(times) // 2], times


_BASELINE_NC = None


def _baseline_program():
    """Minimal program: one small DMA in/out. Measures dispatch overhead."""
    global _BASELINE_NC
    if _BASELINE_NC is not None:
        return _BASELINE_NC
    nc = bacc.Bacc("TRN2", target_bir_lowering=False, debug=False,
                   num_devices=N_CORES)
    a_d = nc.dram_tensor("a", [128, 16], F32, kind="ExternalInput").ap()
    b_d = nc.dram_tensor("b", [128, 16], F32, kind="ExternalOutput").ap()
    with TileContext(nc) as tc, ExitStack() as ctx:
        p = ctx.enter_context(tc.tile_pool(name="p", bufs=1))
        t = p.tile([128, 16], F32)
        nc.gpsimd.dma_start(out=t[:], in_=a_d)
        nc.gpsimd.dma_start(out=b_d, in_=t[:])
    nc.compile()
    _BASELINE_NC = nc
    return nc


def timed_run(inputs, iters=12):
    global _CACHED_NC
    if _CACHED_NC is None:
        _CACHED_NC = _build_program()
    nc = _CACHED_NC
    x = np.asarray(inputs["x"], np.float32)
    wmap = _prep_weights(*[np.asarray(inputs[k], np.float32) for k in
                           ["conv1_w", "conv1_b", "prim_w", "prim_b",
                            "dig_W", "dig_Wb", "out_w", "out_b"]])
    xs = x.reshape(N_CORES, BD, 784)
    in_maps = [dict(x=np.ascontiguousarray(xs[i]), **wmap)
               for i in range(N_CORES)]
    med, times = _time_calls(nc, in_maps, iters)

    bnc = _baseline_program()
    bmaps = [dict(a=np.zeros((128, 16), np.float32)) for _ in range(N_CORES)]
    bmed, btimes = _time_calls(bnc, bmaps, iters)

    ns = (med - bmed) * 1e9
    print(f"kernel call med {med*1e3:.2f} ms (min {times[0]*1e3:.2f}), "
          f"baseline med {bmed*1e3:.2f} ms (min {btimes[0]*1e3:.2f})")
    print(f"per-exec estimate {ns:.0f} ns")
    return int(ns)

